# revision 14
# baseline (speedup 1.0000x reference)
"""Locally-connected autoencoder (128 independent 256->8->256 per-patch linears
+ sigmoid) on 8 Trainium2 NeuronCores.

Strategy
--------
Feature-parallel: core k owns image-row bands (2k, 2k+1) — each band is 16
image rows x 128 cols = one row of 8 patches — for ALL 2048 samples.  The
host ships x pre-transposed (features on partitions) so the per-patch GEMMs
contract over SBUF partitions with zero on-chip transposes.

Everything rides in fp16 (10-bit mantissa): x ~ N(0,1) never needs bf16's
exponent range, and fp16 matmuls stream at the full bf16 rate on TRN2.  The
whole error budget (~4e-4 per tensor) lands well under the 2e-2 gate, with
HALF the HBM traffic of the fp32 baseline on both input and output — the
kernel is DMA-bound, so bytes are the score.

The autoencoder is linear up to the final sigmoid, so the encode bias
folds into the decode bias on the host: bd' = bd + Wd @ be.  The latent
stage is then a pure PSUM->SBUF fp16 copy, done by the otherwise-idle DVE,
keeping ACT free to do only the 8.4M-element sigmoid stream.

Per band (16 chunks of 512 samples x 2 bands per core):
  encode:  z[64(pw,h), n512] = sum_r We_bd[r].T @ X[r]  (16 fp16 matmuls
           accumulated in PSUM; We_bd is the block-diag arrangement of We)
  latent:  DVE copies z PSUM -> SBUF fp16.
  decode (transposed): out^T[128 f, n512] = Wd_bd[:, fchunk].T @ z
           (1 fp16 matmul per fchunk, K=64); fchunk = one image row r'.
  sigmoid: ACT reads 2-bank decode PSUM -> fp16 out^T tiles (+bd' bias)
           -> DMA to HBM.
The host re-transposes the per-core out^T back to [n, f] and upcasts.
"""

import numpy as np

# problem constants (hardcoded per contract)
H, W, PS = 256, 128, 16
NPH, NPW = H // PS, W // PS      # 16 bands, 8 patches/band
P, D, HID = NPH * NPW, PS * PS, 8
NSMP = 4 * 512                    # 2048 samples
BANDW = PS * W                    # 2048 floats per band per sample
NCORES = 8
BPC = NPH // NCORES               # 2 bands per core
M = NPW * HID                     # 64 latent rows per band
NT = 4                            # sample tiles per band
NW = NSMP // NT                   # 512 samples per tile

_PROG = None
LAST_EXEC_NS = None   # filled when kernel() runs with _trace=True
LAST_RES = None       # full BassKernelResults from the last traced run


def _install_ntff_hook():
    """The agent image's antenv lacks axon_hooks; synthesize it so
    run_bass_kernel_spmd(trace=True) can capture NTFF profiles."""
    import sys, types
    try:
        import antenv.axon_hooks  # noqa: F401
        return
    except ImportError:
        pass
    try:
        from trn_agent_boot.trn_boot import _ntff_profile_via_ctypes
        hook = _ntff_profile_via_ctypes('/opt/axon/libaxon_pjrt.so')
    except Exception:
        hook = None
    import antenv
    mod = types.ModuleType("antenv.axon_hooks")
    mod.get_axon_ntff_profile_hook = lambda: hook
    mod.set_axon_ntff_profile_hook = lambda h: None
    antenv.axon_hooks = mod
    sys.modules["antenv.axon_hooks"] = mod


def _patch_tile_drain():
    """This image's walrus caps instructions at ONE sync wait.  Tile attaches
    one wait per outstanding semaphore to the exit drain and can give body
    instructions several waits.  Split: hoist all but one wait onto fresh
    single-wait NOPs inserted immediately before, on the same engine (engine
    streams are in-order, so this is semantics-preserving)."""
    import concourse.tile as tile
    import bass_rust
    from concourse.vector_clock import ScopedClock

    if getattr(tile.TileContext, "_drain_split_patched", False):
        return

    def patched(self, tick_clock, wait_clock):
        drain_inst = self.nc.sync.drain()
        wait_clock.add_sem_waits(
            drain_inst.ins, ScopedClock({None: tick_clock.global_clock})
        )
        si = drain_inst.ins.sync_info
        w = si.on_wait if si else []
        if len(w) > 1:
            drain_inst.ins.sync_info.on_wait = w[:1]
            for x in w[1:]:
                d2 = self.nc.sync.drain()
                d2.ins.sync_info = bass_rust.SyncInfo(on_wait=[x], on_update=[])
        self.nc.all_engine_barrier()
        assert self.sems is not None
        popped = self.nc._tile_sem_poison_stack.pop()
        assert popped is self._sem_poison
        self.nc.clear_and_free_semaphores(list(self.sems.allocated().values()))
        self.nc.all_engine_barrier()

    tile.TileContext._drain_and_barrier = patched

    from concourse import mybir
    from concourse.tile_scheduler import BassTileLoopBlock, BassTileRelease

    _special = [BassTileLoopBlock, BassTileRelease]
    for nm in ("BassTileCriticalSection", "BassTileBranchHintPlaceholder",
               "TileBranchInst", "BassTileConditionalBlock"):
        cls = getattr(tile, nm, None)
        if cls is not None:
            _special.append(cls)
    _special = tuple(_special)

    orig_lower = tile.TileContext._lower_ordered_insts

    def patched_lower(self, ordered):
        for bb_name in list(ordered.keys()):
            insts = ordered[bb_name]
            if not any(
                i.sync_info is not None and len(i.sync_info.on_wait) > 1
                for i in insts
            ):
                continue
            new = []
            for inst in insts:
                si = inst.sync_info
                if (
                    si is not None
                    and len(si.on_wait) > 1
                    and not isinstance(inst, _special)
                ):
                    waits = list(si.on_wait)
                    for x in waits[:-1]:
                        nop = mybir.InstNoOp(
                            name=self.nc.get_next_instruction_name(),
                            ins=[],
                            outs=[],
                            engine=inst.engine,
                            bass_nofuse=True,
                            sync_info=bass_rust.SyncInfo(on_wait=[x], on_update=[]),
                        )
                        new.append(nop)
                    si.on_wait = waits[-1:]
                new.append(inst)
            ordered[bb_name] = new
        return orig_lower(self, ordered)

    tile.TileContext._lower_ordered_insts = patched_lower
    tile.TileContext._drain_split_patched = True


def _build_program():
    """Build the per-core Bass program (same program for all 8 cores)."""
    global _PROG
    if _PROG is not None:
        return _PROG

    import concourse.bass as bass
    import concourse.tile as tile
    from concourse import mybir

    _patch_tile_drain()

    f32 = mybir.dt.float32
    f16 = mybir.dt.float16
    AFT = mybir.ActivationFunctionType

    nc = bass.Bass("TRN2", target_bir_lowering=False, debug=False)

    # host pre-arranges every tensor partition-major and contiguous
    xt_d = nc.dram_tensor("xt", [BPC, 2, W, 2, PS, NW], f16, kind="ExternalInput").ap()
    we_d = nc.dram_tensor("we", [W, BPC, PS, M], f16, kind="ExternalInput").ap()
    wd_d = nc.dram_tensor("wd", [M, BPC, BANDW], f16, kind="ExternalInput").ap()
    bdv_d = nc.dram_tensor("bdv", [W, BPC, PS], f32, kind="ExternalInput").ap()
    out_d = nc.dram_tensor("out", [BPC, PS, W, NSMP], f16, kind="ExternalOutput").ap()

    with tile.TileContext(nc) as tc:
        with (
            tc.tile_pool(name="singles", bufs=1) as singles,
            tc.tile_pool(name="xp", bufs=4) as xpool,
            tc.tile_pool(name="zhp", bufs=6) as zhpool,
            tc.tile_pool(name="outsb", bufs=22) as opool,
            tc.tile_pool(name="zps", bufs=2, space="PSUM") as zpsum,
            tc.tile_pool(name="ops", bufs=3, space="PSUM") as opsum,
        ):
            # warm-up: HAM throttles the PE to half rate until ~3.4us of
            # dense activity, and the first ACTIVATE pays a ~1.3us table
            # load.  Burn both during the otherwise-dead DMA fill with
            # dummy ops on a scratch tile.
            scratch = singles.tile([W, 256], f16)
            nc.gpsimd.memset(scratch, 0.0)
            warm_ps = opsum.tile([W, 2 * NW], f32, name="o_ps")
            for i in range(48):
                nc.tensor.matmul(warm_ps[:, 0:256], lhsT=scratch[:, :128],
                                 rhs=scratch, start=True, stop=True)
            warm_sb = singles.tile([W, 256], f16)
            nc.scalar.activation(out=warm_sb, in_=warm_ps[:, 0:256],
                                 func=AFT.Sigmoid, scale=1.0)

            # encode weights first: the first matmul needs only these
            we_sb = singles.tile([W, BPC, PS, M], f16)
            nc.sync.dma_start(out=we_sb, in_=we_d)

            x_tiles = {}

            def load_x(b, h):
                X = xpool.tile([W, 2, PS, NW], f16, name="X")
                nc.sync.dma_start(out=X, in_=xt_d[b, h])
                x_tiles[(b, h)] = X

            # all x up-front, one 4MiB DMA per half-band -> the input
            # stream never stalls; wd early enough for the first decode
            load_x(0, 0)

            wd_sb = singles.tile([M, BPC, BANDW], f16)
            nc.sync.dma_start(out=wd_sb, in_=wd_d)
            bdv_sb = singles.tile([W, BPC, PS], f32)
            nc.sync.dma_start(out=bdv_sb, in_=bdv_d)

            load_x(0, 1)
            load_x(1, 0)
            load_x(1, 1)

            # half-band pipeline: g = (band, n-half).  Encode of half-band
            # g+1 is interleaved into the decode fc-loop of g (2 matmuls per
            # fc) so the PE stream stays dense and ACT never waits at a
            # half-band boundary.
            halves = [(b, h) for b in range(BPC) for h in range(2)]

            def encode_mm(g, r):
                """Issue encode matmul row r for both chunks of half-band g."""
                b, h = g
                for c in range(2):
                    nc.tensor.matmul(
                        z_ps_cur[c], lhsT=we_sb[:, b, r, :],
                        rhs=x_tiles[(b, h)][:, c, r, :],
                        start=(r == 0), stop=(r == PS - 1),
                    )

            def encode_finish(g):
                """PSUM -> SBUF fp16 latent copies for half-band g."""
                b, h = g
                zh_pair = []
                for c in range(2):
                    zh = zhpool.tile([M, NW], f16, name="zh")
                    nc.vector.tensor_copy(zh, z_ps_cur[c])
                    zh_pair.append(zh)
                return zh_pair

            def encode_start():
                return [zpsum.tile([M, NW], f32, name="z_ps") for _ in range(2)]

            # prologue: encode half-band 0 densely
            z_ps_cur = encode_start()
            for r in range(PS):
                encode_mm(halves[0], r)
            zh_cur = encode_finish(halves[0])

            for gi, g in enumerate(halves):
                b, h = g
                nxt = halves[gi + 1] if gi + 1 < len(halves) else None
                if nxt is not None:
                    z_ps_cur = encode_start()
                # keep ACT free of DMA issues: even half-bands ride the idle
                # GpSimd's SWDGE, odd ones the sync ring (inputs done by then)
                out_eng = nc.gpsimd if gi % 2 == 0 else nc.sync
                zh_pair = zh_cur
                for fc in range(PS):
                    o_ps = opsum.tile([W, 2 * NW], f32, name="o_ps")
                    wstat = wd_sb[:, b, fc * W:(fc + 1) * W]
                    for c in range(2):
                        nc.tensor.matmul(
                            o_ps[:, NW * c:NW * (c + 1)],
                            lhsT=wstat, rhs=zh_pair[c],
                            start=True, stop=True,
                        )
                    if nxt is not None:
                        encode_mm(nxt, fc)
                    o_sb = opool.tile([W, 2 * NW], f16, name="o_sb")
                    nc.scalar.activation(
                        out=o_sb, in_=o_ps, func=AFT.Sigmoid,
                        bias=bdv_sb[:, b, fc:fc + 1], scale=1.0,
                    )
                    out_eng.dma_start(
                        out=out_d[b, fc, :, 2 * NW * h:2 * NW * (h + 1)],
                        in_=o_sb,
                    )
                if nxt is not None:
                    zh_cur = encode_finish(nxt)

    _PROG = nc
    return nc


def _host_prep(x, We, be, Wd, bd):
    """Slice/transpose inputs into per-core maps (pure numpy)."""
    x = np.ascontiguousarray(np.asarray(x, dtype=np.float32)).reshape(NSMP, H * W)
    We = np.asarray(We, dtype=np.float32)
    be = np.asarray(be, dtype=np.float32)
    Wd = np.asarray(Wd, dtype=np.float32)
    bd = np.asarray(bd, dtype=np.float32)

    xT = np.ascontiguousarray(x.T).astype(np.float16)    # [32768, 2048]

    # encode block-diag: wenc[ph, r, 16pw+c, 8pw+h] = We[ph*8+pw, h, r*16+c]
    We6 = We.reshape(NPH, NPW, HID, PS, PS)              # [ph, pw, h, r, c]
    wenc = np.zeros((NPH, PS, W, M), dtype=np.float16)
    for pw in range(NPW):
        wenc[:, :, PS * pw:PS * (pw + 1), HID * pw:HID * (pw + 1)] = (
            We6[:, pw].transpose(0, 2, 3, 1)             # [ph, r, c, h]
        )

    # decode rhs: wdec[ph, 8pw+h, 128r'+16pw+c'] = Wd[ph*8+pw, r'*16+c', h]
    Wd5 = Wd.reshape(NPH, NPW, PS, PS, HID)              # [ph, pw, r', c', h]
    wdec = np.zeros((NPH, M, BANDW), dtype=np.float16)
    wdec_v = wdec.reshape(NPH, NPW, HID, PS, NPW, PS)
    for pw in range(NPW):
        wdec_v[:, pw, :, :, pw, :] = Wd5[:, pw].transpose(0, 3, 1, 2)  # [ph, h, r', c']

    # linear up to the sigmoid -> fold be into the decode bias:
    # bd' = bd + Wd @ be   (per patch)
    bdf = bd + np.einsum('pdh,ph->pd', Wd, be)
    # per-partition for the transposed output: bdv[w', ph, r']
    bd4 = bdf.reshape(NPH, NPW, PS, PS)                  # [ph, pw, r', c']
    bdv = np.ascontiguousarray(
        bd4.transpose(1, 3, 0, 2).reshape(W, NPH, PS))   # [16pw+c', ph, r']

    in_maps = []
    for k in range(NCORES):
        # xt6[b, h, w, c, r, n] -- per-partition contiguous 32 KiB tiles,
        # one 4 MiB DMA per half-band h (chunks c = 2h+{0,1})
        xc = xT[BPC * BANDW * k: BPC * BANDW * (k + 1)]
        xt5 = np.ascontiguousarray(
            xc.reshape(BPC, PS, W, 2, 2, NW).transpose(0, 3, 2, 4, 1, 5))
        in_maps.append({
            "xt": xt5,
            "we": np.ascontiguousarray(
                wenc[BPC * k:BPC * (k + 1)].transpose(2, 0, 1, 3)),
            "wd": np.ascontiguousarray(
                wdec[BPC * k:BPC * (k + 1)].transpose(1, 0, 2)),
            "bdv": np.ascontiguousarray(bdv[:, BPC * k:BPC * (k + 1), :]),
        })
    return in_maps


def kernel(x, We, be, Wd, bd, _trace=False):
    global LAST_EXEC_NS
    from concourse.bass_utils import run_bass_kernel_spmd

    if _trace:
        _install_ntff_hook()

    nc = _build_program()
    in_maps = _host_prep(x, We, be, Wd, bd)
    res = run_bass_kernel_spmd(nc, in_maps, list(range(NCORES)), trace=_trace)
    if _trace:
        LAST_EXEC_NS = res.exec_time_ns
        global LAST_RES
        LAST_RES = res

    # out_k is out^T: [band, r'(=fchunk), w', n]  ->  out[n, band*2048 + 128 r' + w']
    cols = [
        np.asarray(res.results[k]["out"]).reshape(BPC * BANDW, NSMP).T
        for k in range(NCORES)
    ]
    out = np.concatenate(cols, axis=1).astype(np.float32)
    return np.ascontiguousarray(out.reshape(4, 512, H * W))


# revision 19
# speedup vs baseline: 1.1738x; 1.1738x over previous
"""Locally-connected autoencoder (128 independent 256->8->256 per-patch linears
+ sigmoid) on 8 Trainium2 NeuronCores.

Strategy
--------
Feature-parallel: core k owns image-row bands (2k, 2k+1) — each band is 16
image rows x 128 cols = one row of 8 patches — for ALL 2048 samples.  The
host ships x pre-transposed (features on partitions) so the per-patch GEMMs
contract over SBUF partitions with zero on-chip transposes.

Everything rides in fp16 (10-bit mantissa): x ~ N(0,1) never needs bf16's
exponent range, and fp16 matmuls stream at the full bf16 rate on TRN2.  The
whole error budget (~4e-4 per tensor) lands well under the 2e-2 gate, with
HALF the HBM traffic of the fp32 baseline on both input and output — the
kernel is DMA-bound, so bytes are the score.

The autoencoder is linear up to the final sigmoid, so the encode bias
folds into the decode bias on the host: bd' = bd + Wd @ be.  The latent
stage is then a pure PSUM->SBUF fp16 copy, done by the otherwise-idle DVE,
keeping ACT free to do only the 8.4M-element sigmoid stream.

Per band (16 chunks of 512 samples x 2 bands per core):
  encode:  z[64(pw,h), n512] = sum_r We_bd[r].T @ X[r]  (16 fp16 matmuls
           accumulated in PSUM; We_bd is the block-diag arrangement of We)
  latent:  DVE copies z PSUM -> SBUF fp16.
  decode (transposed): out^T[128 f, n512] = Wd_bd[:, fchunk].T @ z
           (1 fp16 matmul per fchunk, K=64); fchunk = one image row r'.
  sigmoid: ACT reads 2-bank decode PSUM -> fp16 out^T tiles (+bd' bias)
           -> DMA to HBM.
The host re-transposes the per-core out^T back to [n, f] and upcasts.
"""

import numpy as np

# problem constants (hardcoded per contract)
H, W, PS = 256, 128, 16
NPH, NPW = H // PS, W // PS      # 16 bands, 8 patches/band
P, D, HID = NPH * NPW, PS * PS, 8
NSMP = 4 * 512                    # 2048 samples
BANDW = PS * W                    # 2048 floats per band per sample
NCORES = 8
BPC = NPH // NCORES               # 2 bands per core
M = NPW * HID                     # 64 latent rows per band
NT = 4                            # sample tiles per band
NW = NSMP // NT                   # 512 samples per tile

_PROG = None
LAST_EXEC_NS = None   # filled when kernel() runs with _trace=True
LAST_RES = None       # full BassKernelResults from the last traced run


def _install_ntff_hook():
    """The agent image's antenv lacks axon_hooks; synthesize it so
    run_bass_kernel_spmd(trace=True) can capture NTFF profiles."""
    import sys, types
    try:
        import antenv.axon_hooks  # noqa: F401
        return
    except ImportError:
        pass
    try:
        from trn_agent_boot.trn_boot import _ntff_profile_via_ctypes
        hook = _ntff_profile_via_ctypes('/opt/axon/libaxon_pjrt.so')
    except Exception:
        hook = None
    import antenv
    mod = types.ModuleType("antenv.axon_hooks")
    mod.get_axon_ntff_profile_hook = lambda: hook
    mod.set_axon_ntff_profile_hook = lambda h: None
    antenv.axon_hooks = mod
    sys.modules["antenv.axon_hooks"] = mod


def _patch_tile_drain():
    """This image's walrus caps instructions at ONE sync wait.  Tile attaches
    one wait per outstanding semaphore to the exit drain and can give body
    instructions several waits.  Split: hoist all but one wait onto fresh
    single-wait NOPs inserted immediately before, on the same engine (engine
    streams are in-order, so this is semantics-preserving)."""
    import concourse.tile as tile
    import bass_rust
    from concourse.vector_clock import ScopedClock

    if getattr(tile.TileContext, "_drain_split_patched", False):
        return

    def patched(self, tick_clock, wait_clock):
        drain_inst = self.nc.sync.drain()
        wait_clock.add_sem_waits(
            drain_inst.ins, ScopedClock({None: tick_clock.global_clock})
        )
        si = drain_inst.ins.sync_info
        w = si.on_wait if si else []
        if len(w) > 1:
            drain_inst.ins.sync_info.on_wait = w[:1]
            for x in w[1:]:
                d2 = self.nc.sync.drain()
                d2.ins.sync_info = bass_rust.SyncInfo(on_wait=[x], on_update=[])
        self.nc.all_engine_barrier()
        assert self.sems is not None
        popped = self.nc._tile_sem_poison_stack.pop()
        assert popped is self._sem_poison
        self.nc.clear_and_free_semaphores(list(self.sems.allocated().values()))
        self.nc.all_engine_barrier()

    tile.TileContext._drain_and_barrier = patched

    from concourse import mybir
    from concourse.tile_scheduler import BassTileLoopBlock, BassTileRelease

    _special = [BassTileLoopBlock, BassTileRelease]
    for nm in ("BassTileCriticalSection", "BassTileBranchHintPlaceholder",
               "TileBranchInst", "BassTileConditionalBlock"):
        cls = getattr(tile, nm, None)
        if cls is not None:
            _special.append(cls)
    _special = tuple(_special)

    orig_lower = tile.TileContext._lower_ordered_insts

    def patched_lower(self, ordered):
        for bb_name in list(ordered.keys()):
            insts = ordered[bb_name]
            if not any(
                i.sync_info is not None and len(i.sync_info.on_wait) > 1
                for i in insts
            ):
                continue
            new = []
            for inst in insts:
                si = inst.sync_info
                if (
                    si is not None
                    and len(si.on_wait) > 1
                    and not isinstance(inst, _special)
                ):
                    waits = list(si.on_wait)
                    for x in waits[:-1]:
                        nop = mybir.InstNoOp(
                            name=self.nc.get_next_instruction_name(),
                            ins=[],
                            outs=[],
                            engine=inst.engine,
                            bass_nofuse=True,
                            sync_info=bass_rust.SyncInfo(on_wait=[x], on_update=[]),
                        )
                        new.append(nop)
                    si.on_wait = waits[-1:]
                new.append(inst)
            ordered[bb_name] = new
        return orig_lower(self, ordered)

    tile.TileContext._lower_ordered_insts = patched_lower
    tile.TileContext._drain_split_patched = True


def _build_program():
    """Build the per-core Bass program (same program for all 8 cores)."""
    global _PROG
    if _PROG is not None:
        return _PROG

    import concourse.bass as bass
    import concourse.tile as tile
    from concourse import mybir

    _patch_tile_drain()

    f32 = mybir.dt.float32
    f16 = mybir.dt.float16
    AFT = mybir.ActivationFunctionType

    nc = bass.Bass("TRN2", target_bir_lowering=False, debug=False)

    # host pre-arranges every tensor partition-major and contiguous
    xt_d = nc.dram_tensor("xt", [BPC, NT, W, PS, NW], f16, kind="ExternalInput").ap()
    we_d = nc.dram_tensor("we", [W, BPC, PS, M], f16, kind="ExternalInput").ap()
    wd_d = nc.dram_tensor("wd", [M, BPC, BANDW], f16, kind="ExternalInput").ap()
    bdv_d = nc.dram_tensor("bdv", [W, BPC, PS], f32, kind="ExternalInput").ap()
    out_d = nc.dram_tensor("out", [BPC, PS, W, NSMP], f16, kind="ExternalOutput").ap()

    with tile.TileContext(nc) as tc:
        with (
            tc.tile_pool(name="singles", bufs=1) as singles,
            tc.tile_pool(name="xp", bufs=2 * NT) as xpool,
            tc.tile_pool(name="zhp", bufs=6) as zhpool,
            tc.tile_pool(name="outsb", bufs=22) as opool,
            tc.tile_pool(name="zps", bufs=2, space="PSUM") as zpsum,
            tc.tile_pool(name="ops", bufs=3, space="PSUM") as opsum,
        ):
            # warm-up: HAM throttles the PE to half rate until ~3.4us of
            # dense activity, and the first ACTIVATE pays a ~1.3us table
            # load.  Burn both during the otherwise-dead DMA fill with
            # dummy ops on a scratch tile.
            scratch = singles.tile([W, 256], f16)
            nc.gpsimd.memset(scratch, 0.0)
            warm_ps = opsum.tile([W, 2 * NW], f32, name="o_ps")
            for i in range(48):
                nc.tensor.matmul(warm_ps[:, 0:256], lhsT=scratch[:, :128],
                                 rhs=scratch, start=True, stop=True)
            warm_sb = singles.tile([W, 256], f16)
            nc.scalar.activation(out=warm_sb, in_=warm_ps[:, 0:256],
                                 func=AFT.Sigmoid, scale=1.0)

            # encode weights first: the first matmul needs only these
            we_sb = singles.tile([W, BPC, PS, M], f16)
            nc.sync.dma_start(out=we_sb, in_=we_d)

            x_tiles = {}

            def load_x(b, t):
                X = xpool.tile([W, PS, NW], f16, name="X")
                nc.sync.dma_start(out=X, in_=xt_d[b, t])
                x_tiles[(b, t)] = X

            # all x up-front (pool holds all 8 tiles -> the input stream
            # never stalls); wd slots in early enough for the first decode
            load_x(0, 0)
            load_x(0, 1)

            wd_sb = singles.tile([M, BPC, BANDW], f16)
            nc.sync.dma_start(out=wd_sb, in_=wd_d)
            bdv_sb = singles.tile([W, BPC, PS], f32)
            nc.sync.dma_start(out=bdv_sb, in_=bdv_d)

            for b in range(BPC):
                for t in range(NT):
                    if (b, t) not in x_tiles:
                        load_x(b, t)

            # half-band pipeline: g = (band, n-half).  Encode of half-band
            # g+1 is interleaved into the decode fc-loop of g (2 matmuls per
            # fc) so the PE stream stays dense and ACT never waits at a
            # half-band boundary.
            halves = [(b, h) for b in range(BPC) for h in range(2)]

            def encode_mm(g, r):
                """Issue encode matmul row r for both chunks of half-band g."""
                b, h = g
                for c in range(2):
                    nc.tensor.matmul(
                        z_ps_cur[c], lhsT=we_sb[:, b, r, :],
                        rhs=x_tiles[(b, 2 * h + c)][:, r, :],
                        start=(r == 0), stop=(r == PS - 1),
                    )

            def encode_finish(g):
                """PSUM -> SBUF fp16 latent copies for half-band g."""
                b, h = g
                zh_pair = []
                for c in range(2):
                    zh = zhpool.tile([M, NW], f16, name="zh")
                    nc.vector.tensor_copy(zh, z_ps_cur[c])
                    zh_pair.append(zh)
                return zh_pair

            def encode_start():
                return [zpsum.tile([M, NW], f32, name="z_ps") for _ in range(2)]

            # prologue: encode half-band 0 densely
            z_ps_cur = encode_start()
            for r in range(PS):
                encode_mm(halves[0], r)
            zh_cur = encode_finish(halves[0])

            for gi, g in enumerate(halves):
                b, h = g
                nxt = halves[gi + 1] if gi + 1 < len(halves) else None
                if nxt is not None:
                    z_ps_cur = encode_start()
                # keep ACT free of DMA issues: even half-bands ride the idle
                # GpSimd's SWDGE, odd ones the sync ring (inputs done by then)
                out_eng = nc.gpsimd if gi % 2 == 0 else nc.sync
                zh_pair = zh_cur
                for fc in range(PS):
                    o_ps = opsum.tile([W, 2 * NW], f32, name="o_ps")
                    wstat = wd_sb[:, b, fc * W:(fc + 1) * W]
                    for c in range(2):
                        nc.tensor.matmul(
                            o_ps[:, NW * c:NW * (c + 1)],
                            lhsT=wstat, rhs=zh_pair[c],
                            start=True, stop=True,
                        )
                    if nxt is not None:
                        encode_mm(nxt, fc)
                    o_sb = opool.tile([W, 2 * NW], f16, name="o_sb")
                    nc.scalar.activation(
                        out=o_sb, in_=o_ps, func=AFT.Sigmoid,
                        bias=bdv_sb[:, b, fc:fc + 1], scale=1.0,
                    )
                    out_eng.dma_start(
                        out=out_d[b, fc, :, 2 * NW * h:2 * NW * (h + 1)],
                        in_=o_sb,
                    )
                if nxt is not None:
                    zh_cur = encode_finish(nxt)

    _PROG = nc
    return nc


def _host_prep(x, We, be, Wd, bd):
    """Slice/transpose inputs into per-core maps (pure numpy)."""
    x = np.ascontiguousarray(np.asarray(x, dtype=np.float32)).reshape(NSMP, H * W)
    We = np.asarray(We, dtype=np.float32)
    be = np.asarray(be, dtype=np.float32)
    Wd = np.asarray(Wd, dtype=np.float32)
    bd = np.asarray(bd, dtype=np.float32)

    xT = np.ascontiguousarray(x.T).astype(np.float16)    # [32768, 2048]

    # encode block-diag: wenc[ph, r, 16pw+c, 8pw+h] = We[ph*8+pw, h, r*16+c]
    We6 = We.reshape(NPH, NPW, HID, PS, PS)              # [ph, pw, h, r, c]
    wenc = np.zeros((NPH, PS, W, M), dtype=np.float16)
    for pw in range(NPW):
        wenc[:, :, PS * pw:PS * (pw + 1), HID * pw:HID * (pw + 1)] = (
            We6[:, pw].transpose(0, 2, 3, 1)             # [ph, r, c, h]
        )

    # decode rhs: wdec[ph, 8pw+h, 128r'+16pw+c'] = Wd[ph*8+pw, r'*16+c', h]
    Wd5 = Wd.reshape(NPH, NPW, PS, PS, HID)              # [ph, pw, r', c', h]
    wdec = np.zeros((NPH, M, BANDW), dtype=np.float16)
    wdec_v = wdec.reshape(NPH, NPW, HID, PS, NPW, PS)
    for pw in range(NPW):
        wdec_v[:, pw, :, :, pw, :] = Wd5[:, pw].transpose(0, 3, 1, 2)  # [ph, h, r', c']

    # linear up to the sigmoid -> fold be into the decode bias:
    # bd' = bd + Wd @ be   (per patch)
    bdf = bd + np.einsum('pdh,ph->pd', Wd, be)
    # per-partition for the transposed output: bdv[w', ph, r']
    bd4 = bdf.reshape(NPH, NPW, PS, PS)                  # [ph, pw, r', c']
    bdv = np.ascontiguousarray(
        bd4.transpose(1, 3, 0, 2).reshape(W, NPH, PS))   # [16pw+c', ph, r']

    in_maps = []
    for k in range(NCORES):
        # xt5[b, t, w, r, n] -- per-partition contiguous 16 KiB tiles
        xc = xT[BPC * BANDW * k: BPC * BANDW * (k + 1)]
        xt5 = np.ascontiguousarray(
            xc.reshape(BPC, PS, W, NT, NW).transpose(0, 3, 2, 1, 4))
        in_maps.append({
            "xt": xt5,
            "we": np.ascontiguousarray(
                wenc[BPC * k:BPC * (k + 1)].transpose(2, 0, 1, 3)),
            "wd": np.ascontiguousarray(
                wdec[BPC * k:BPC * (k + 1)].transpose(1, 0, 2)),
            "bdv": np.ascontiguousarray(bdv[:, BPC * k:BPC * (k + 1), :]),
        })
    return in_maps


def kernel(x, We, be, Wd, bd, _trace=False):
    global LAST_EXEC_NS
    from concourse.bass_utils import run_bass_kernel_spmd

    if _trace:
        _install_ntff_hook()

    nc = _build_program()
    in_maps = _host_prep(x, We, be, Wd, bd)
    res = run_bass_kernel_spmd(nc, in_maps, list(range(NCORES)), trace=_trace)
    if _trace:
        LAST_EXEC_NS = res.exec_time_ns
        global LAST_RES
        LAST_RES = res

    # out_k is out^T: [band, r'(=fchunk), w', n]  ->  out[n, band*2048 + 128 r' + w']
    cols = [
        np.asarray(res.results[k]["out"]).reshape(BPC * BANDW, NSMP).T
        for k in range(NCORES)
    ]
    out = np.concatenate(cols, axis=1).astype(np.float32)
    return np.ascontiguousarray(out.reshape(4, 512, H * W))
